# revision 39
# baseline (speedup 1.0000x reference)
"""BatchMultiHeadGraphAttention TRN2 kernel (fp8 DoubleRow PV).

Reference computation (per batch b, head h):
    h_prime = h[b] @ w[h]                          # [n, f]
    t = tanh(h_prime)
    src = t @ a_src[h];  dst = t @ a_dst[h]        # [n]
    s[i, j] = leaky_relu(src[i] + dst[j], 0.2)
    s = where(adj[b] | eye, s, -inf)
    attn = softmax(s, axis=-1)
    out[b, h] = attn @ h_prime + bias

Sharding: 8 cores, one (b, h) slab per core (bs=4 x H=2).

Algebra: exp(lrelu(s)) = v_j * max(u_i, p_i * r_j) with u=exp(src),
v=exp(dst), p=exp(0.2 src), r=exp(-0.8 dst).  The v_j factor is folded
into the value matrix (hpz = v_j * [h_prime | 1], the extra column
computing the softmax denominator Z), so softmax(s) @ h_prime =
(P @ hpz)[:, :F] / (P @ hpz)[:, F] with P[j, i] = mask * max(u_i, p_i r_j).

fp8 PV: P and hpz are held as e4m3 hi+residual pairs and the PV matmul
runs in MatmulPerfMode.DoubleRow (2 contraction tiles per instruction at
0.5 cycles/row -> 4x bf16 PE throughput).  Three DoubleRow passes,
P8@hi + P8@lo + dP8@hi, recover bf16-level accuracy (2.8e-3 max rel):
per-row scales c_i = 216/max(u_i, p_i*rmax_bound) (softmax-invariant)
put P in e4m3 range, a global g=4 scale (softmax-invariant, folded so
the hp scale is exactly exp(dst)) puts hpz in range.

Host-side prep does all layout work: h arrives pre-transposed in bf16
(hT), w in bf16, and the adjacency arrives transposed, diagonal-ORed,
column-permuted to the on-chip i order, and encoded as u16 0xFFFF/0 --
so masking P8/dP8 is a single 2x-mode bitwise-AND over a u16 plane that
holds both fp8 planes byte-interleaved.

On-chip i order within group g is (p, t)-major: chip i = 128*HT*g +
p*HT + t  <->  logical i = 128*(HT*g + t) + p.  This makes the
broadcast u/p rows plain contiguous DMAs; only the final output DMA
un-permutes (same descriptor count).
"""

import numpy as np

BS, N, H, F = 4, 2048, 2, 768
NCORES = 8
NT = N // 128          # 16 j tiles (tau)
KT = F // 128          # 6 k tiles
NG = 2                 # i groups
GW = N // NG           # 1024
HT = NT // NG          # 8 taus per group
CHG = GW // 128        # 8 i-chunks per group
HPW8 = 772             # fp8 value row stride (768 + Z col + pad)
LN216 = float(np.log(216.0))
LNRB = float(np.log(64.0))   # upper bound on ln(max_j r_j); true <= ln(18)

_CACHE = {}


def _chip_perm():
    """chip i -> logical i permutation (length N)."""
    perm = np.empty(N, np.int64)
    for g in range(NG):
        ci = np.arange(GW)
        perm[g * GW + ci] = 128 * (HT * g + ci % HT) + ci // HT
    return perm


def _build(has_bias: bool):
    import os
    import concourse.bass as bass
    import concourse.mybir as mybir
    import concourse.tile as tile
    from concourse import bacc

    dt = mybir.dt
    AF = mybir.ActivationFunctionType
    OP = mybir.AluOpType
    PM = mybir.MatmulPerfMode
    dbg = bool(int(os.environ.get("BMGA_DEBUG", "0")))

    nc = bacc.Bacc("TRN2", target_bir_lowering=False, debug=False,
                   num_devices=NCORES)

    d_hT = nc.dram_tensor("hT", [F, N], dt.bfloat16, kind="ExternalInput")
    d_w = nc.dram_tensor("w", [F, F], dt.bfloat16, kind="ExternalInput")
    d_adjT = nc.dram_tensor("adjT", [N, N], dt.uint16, kind="ExternalInput")
    d_asrc = nc.dram_tensor("a_src", [F], dt.float32, kind="ExternalInput")
    d_adst = nc.dram_tensor("a_dst", [F], dt.float32, kind="ExternalInput")
    if has_bias:
        d_bias = nc.dram_tensor("bias", [F], dt.float32, kind="ExternalInput")
    d_out = nc.dram_tensor("out", [N, F], dt.float32, kind="ExternalOutput")
    if dbg:
        d_dbgc = nc.dram_tensor("dbgc", [128, 4 * NT], dt.float32,
                                kind="ExternalOutput")
        d_dbgPS = nc.dram_tensor("dbgPS", [128, HPW8], dt.float32,
                                 kind="ExternalOutput")
        d_dbgH = nc.dram_tensor("dbgH", [128, 2 * NT * HPW8], dt.uint8,
                                kind="ExternalOutput")
        d_dbgIA = nc.dram_tensor("dbgIA", [128, NT * GW], dt.uint16,
                                 kind="ExternalOutput")

    with tile.TileContext(nc) as tc:
        with tc.tile_pool(name="const", bufs=1) as cpool, \
             tc.tile_pool(name="persist", bufs=1) as pp:
            if has_bias:
                bias_row = cpool.tile([1, F], dt.float32, tag="bias_row")
                nc.sync.dma_start(bias_row[:],
                                  d_bias.ap().rearrange("(o f) -> o f", o=1))
                bias_bc = cpool.tile([128, F], dt.float32, tag="bias_bc")
                nc.gpsimd.partition_broadcast(bias_bc[:], bias_row[:])

            # persistent: fp8 value planes, per-tau j columns, i broadcasts,
            # masked packed (P8, dP8) planes per group
            hi = pp.tile([128, NT * HPW8], dt.float8e4, tag="hi")
            lo = pp.tile([128, NT * HPW8], dt.float8e4, tag="lo")
            src_col = pp.tile([128, NT], dt.float32, tag="src_col")
            dst_col = pp.tile([128, NT], dt.float32, tag="dst_col")
            r_col = pp.tile([128, NT], dt.float32, tag="r_col")
            v_col = pp.tile([128, NT], dt.float32, tag="v_col")
            u_bc = pp.tile([128, N], dt.bfloat16, tag="u_bc")
            p_bc = pp.tile([128, N], dt.bfloat16, tag="p_bc")
            IP = [pp.tile([128, NT * GW], dt.uint16, tag=f"IP{g}",
                          name=f"IP{g}")
                  for g in range(NG)]

            hiv = hi[:].rearrange("p (t f) -> p t f", t=NT)
            lov = lo[:].rearrange("p (t f) -> p t f", t=NT)

            # ---- pools (whole-kernel scope except psum) ----
            with tc.tile_pool(name="ph1", bufs=1) as hpool, \
                 tc.tile_pool(name="ph2", bufs=3) as tpool, \
                 tc.tile_pool(name="scr2", bufs=2) as scrp, \
                 tc.tile_pool(name="hvp", bufs=2) as hvp, \
                 tc.tile_pool(name="p3", bufs=2) as p3, \
                 tc.tile_pool(name="pgm", bufs=6) as mpool, \
                 tc.tile_pool(name="pgsS", bufs=4) as spoolS, \
                 tc.tile_pool(name="pgsD", bufs=4) as spoolD, \
                 tc.tile_pool(name="pgo", bufs=2) as opool:
                # a_src/a_dst rows first: the dots need the broadcasts early
                asrc_row = hpool.tile([1, F], dt.float32, tag="asrc_row")
                nc.sync.dma_start(asrc_row[:],
                                  d_asrc.ap().rearrange("(o f) -> o f", o=1))
                adst_row = hpool.tile([1, F], dt.float32, tag="adst_row")
                nc.sync.dma_start(adst_row[:],
                                  d_adst.ap().rearrange("(o f) -> o f", o=1))
                asrc_bc = hpool.tile([128, F], dt.float32, tag="asrc_bc")
                nc.gpsimd.partition_broadcast(asrc_bc[:], asrc_row[:])
                adst_bc = hpool.tile([128, F], dt.float32, tag="adst_bc")
                nc.gpsimd.partition_broadcast(adst_bc[:], adst_row[:])
                wr = [hpool.tile([128, F], dt.bfloat16, tag=f"wr{k}",
                                 name=f"wr{k}") for k in range(KT)]
                hTt = [hpool.tile([128, N], dt.bfloat16, tag=f"hT{k}",
                                  name=f"hT{k}") for k in range(KT)]
                # k-interleaved w / hT-chunk loads: the k=0 matmul of tau 0
                # can start after the first (w, hT) pair lands
                for k in range(KT):
                    nc.sync.dma_start(wr[k][:], d_w[128 * k:128 * (k + 1), :])
                    nc.sync.dma_start(
                        hTt[k][:, 0:512],
                        d_hT[128 * k:128 * (k + 1), 0:512])
                for chnk in range(1, 4):
                    for k in range(KT):
                        nc.sync.dma_start(
                            hTt[k][:, 512 * chnk:512 * (chnk + 1)],
                            d_hT[128 * k:128 * (k + 1),
                                 512 * chnk:512 * (chnk + 1)])

                Smap, Sdmap, Mmap = {}, {}, {}

                def emit_phase2(tau, psum2):
                    ps = psum2.tile([128, F], dt.float32, tag="hpps")
                    for k in range(KT):
                        lhsT = hTt[k][:, 128 * tau:128 * (tau + 1)]
                        nc.tensor.matmul(ps[:, 0:512], lhsT, wr[k][:, 0:512],
                                         start=(k == 0), stop=(k == KT - 1))
                        nc.tensor.matmul(ps[:, 512:F], lhsT, wr[k][:, 512:F],
                                         start=(k == 0), stop=(k == KT - 1))
                    tnh = tpool.tile([128, F], dt.bfloat16, tag="tnh")
                    nc.scalar.activation(tnh[:], ps[:], AF.Tanh)
                    scrd = scrp.tile([128, F], dt.bfloat16, tag="scr")
                    nc.vector.scalar_tensor_tensor(
                        scrd[:], tnh[:], 1.0, adst_bc[:],
                        op0=OP.mult, op1=OP.mult,
                        accum_out=dst_col[:, tau:tau + 1])
                    nc.scalar.activation(r_col[:, tau:tau + 1],
                                         dst_col[:, tau:tau + 1],
                                         AF.Exp, scale=-0.8)
                    nc.scalar.activation(v_col[:, tau:tau + 1],
                                         dst_col[:, tau:tau + 1], AF.Exp)
                    # value planes: hi = fp8(v * h_prime), lo = residual
                    # (hpv staged in f32 so the lo subtract can run on Pool)
                    hslc = hi[:, HPW8 * tau:HPW8 * tau + F]
                    nc.scalar.activation(hslc, ps[:], AF.Copy,
                                         scale=v_col[:, tau:tau + 1])
                    hpv = hvp.tile([128, F], dt.float32, tag="hpv")
                    if tau % 2 == 1:
                        nc.vector.tensor_scalar(hpv[:], ps[:],
                                                v_col[:, tau:tau + 1], None,
                                                op0=OP.mult)
                    else:
                        nc.scalar.activation(hpv[:], ps[:], AF.Copy,
                                             scale=v_col[:, tau:tau + 1])
                    nc.gpsimd.tensor_tensor(
                        lo[:, HPW8 * tau:HPW8 * tau + F],
                        hpv[:], hslc, op=OP.subtract)
                    scrs = scrp.tile([128, F], dt.bfloat16, tag="scr")
                    nc.vector.scalar_tensor_tensor(
                        scrs[:], tnh[:], 1.0, asrc_bc[:],
                        op0=OP.mult, op1=OP.mult,
                        accum_out=src_col[:, tau:tau + 1])
                    if tau % 4 == 3:
                        hz = hiv[:, tau - 3:tau + 1, F]
                        nc.scalar.activation(hz, v_col[:, tau - 3:tau + 1],
                                             AF.Copy)
                        nc.vector.tensor_tensor(
                            lov[:, tau - 3:tau + 1, F],
                            v_col[:, tau - 3:tau + 1], hz, op=OP.subtract)

                def emit_iside(g):
                    # c_i = 216/max(u_i, p_i*rmax_bound) in log domain
                    t0 = HT * g
                    sg = src_col[:, t0:t0 + HT]
                    s3 = p3.tile([128, HT], dt.float32, tag="s3")
                    nc.vector.tensor_scalar(s3[:], sg, 0.2, None, op0=OP.mult)
                    s2 = p3.tile([128, HT], dt.float32, tag="s2")
                    nc.vector.tensor_scalar(s2[:], s3[:], LNRB, None,
                                            op0=OP.add)
                    mxs = p3.tile([128, HT], dt.float32, tag="mxs")
                    nc.vector.tensor_tensor(mxs[:], sg, s2[:], op=OP.max)
                    nc.vector.tensor_scalar(mxs[:], mxs[:], LN216, None,
                                            op0=OP.subtract)
                    eu = p3.tile([128, HT], dt.float32, tag="eu")
                    nc.vector.tensor_tensor(eu[:], sg, mxs[:],
                                            op=OP.subtract)
                    ep = p3.tile([128, HT], dt.float32, tag="ep")
                    nc.vector.tensor_tensor(ep[:], s3[:], mxs[:],
                                            op=OP.subtract)
                    ux = p3.tile([128, HT], dt.bfloat16, tag="ux")
                    nc.scalar.activation(ux[:], eu[:], AF.Exp)
                    px = p3.tile([128, HT], dt.bfloat16, tag="px")
                    nc.scalar.activation(px[:], ep[:], AF.Exp)
                    u_row = p3.tile([1, GW], dt.bfloat16, tag="u_row")
                    nc.sync.dma_start(
                        u_row[:].rearrange("o (p t) -> o p t", p=128), ux[:])
                    p_row = p3.tile([1, GW], dt.bfloat16, tag="p_row")
                    nc.sync.dma_start(
                        p_row[:].rearrange("o (p t) -> o p t", p=128), px[:])
                    nc.gpsimd.partition_broadcast(
                        u_bc[:, GW * g:GW * (g + 1)], u_row[:])
                    nc.gpsimd.partition_broadcast(
                        p_bc[:, GW * g:GW * (g + 1)], p_row[:])

                def emit_smax(g, tau):
                    mtile = mpool.tile([128, GW], dt.uint16, tag="mask")
                    nc.sync.dma_start(
                        mtile[:],
                        d_adjT[128 * tau:128 * (tau + 1),
                               GW * g:GW * (g + 1)])
                    Mmap[(g, tau)] = mtile
                    S = spoolS.tile([128, GW], dt.float32, tag="S")
                    nc.vector.scalar_tensor_tensor(
                        S[:], p_bc[:, GW * g:GW * (g + 1)],
                        r_col[:, tau:tau + 1],
                        u_bc[:, GW * g:GW * (g + 1)],
                        op0=OP.mult, op1=OP.max)
                    Smap[(g, tau)] = S

                def emit_s8(g, tau):
                    Sd = spoolD.tile([128, GW], dt.uint16, tag="Sd")
                    Sdmap[(g, tau)] = Sd
                    s8v = Sd[:].bitcast(dt.float8e4)[:, 0:2 * GW:2]
                    nc.scalar.activation(s8v, Smap[(g, tau)][:], AF.Copy)

                def emit_ds8(g, tau):
                    Sd = Sdmap[(g, tau)]
                    s8v = Sd[:].bitcast(dt.float8e4)[:, 0:2 * GW:2]
                    ds8v = Sd[:].bitcast(dt.float8e4)[:, 1:2 * GW:2]
                    ti = g * NT + tau
                    ds8eng = nc.vector if ti % 4 == 3 else nc.gpsimd
                    ds8eng.tensor_tensor(ds8v, Smap[(g, tau)][:], s8v,
                                         op=OP.subtract)

                def emit_and(g, tau):
                    nc.vector.tensor_tensor(
                        IP[g][:].bitcast(dt.uint32)[
                            :, GW // 2 * tau:GW // 2 * (tau + 1)],
                        Mmap[(g, tau)][:].bitcast(dt.uint32),
                        Sdmap[(g, tau)][:].bitcast(dt.uint32),
                        op=OP.bitwise_and)
                    del Smap[(g, tau)], Sdmap[(g, tau)], Mmap[(g, tau)]

                # per-slot build tau schedules: g0 trails its last 4 taus
                # into the PV(g0) window so the phase-2 tail window sheds
                # DVE/Pool/ACT work
                SCHED = {0: [(2 * k, 2 * k + 1) for k in range(HT)],
                         1: [(2 * k, 2 * k + 1) for k in range(HT)]}

                def build_slot(g, k):
                    # software-pipelined build: smax@k, S8+dS8@k+1,
                    # AND@k+2
                    if k == 0:
                        emit_iside(g)
                    sched = SCHED[g]
                    for stage, d in ((emit_smax, 0), (emit_s8, 1),
                                     (emit_ds8, 1), (emit_and, 2)):
                        kk = k - d
                        if 0 <= kk < len(sched):
                            for tau in sched[kk]:
                                stage(g, tau)

                with tc.tile_pool(name="ph2ps", bufs=3,
                                  space="PSUM") as psum2:
                    for tau in range(NT):
                        emit_phase2(tau, psum2)
                        if tau >= NT - HT:
                            build_slot(0, tau - (NT - HT))
                    for k in range(HT, HT + 2):
                        build_slot(0, k)
                # remaining g0 build slots trail into the PV block

                if dbg:
                    nc.sync.dma_start(d_dbgc[:, 0:NT], src_col[:])
                    nc.sync.dma_start(d_dbgc[:, NT:2 * NT], dst_col[:])
                    nc.sync.dma_start(d_dbgc[:, 2 * NT:3 * NT], r_col[:])
                    nc.sync.dma_start(d_dbgc[:, 3 * NT:4 * NT], v_col[:])
                    nc.sync.dma_start(d_dbgH[:, 0:NT * HPW8],
                                      hi[:].bitcast(dt.uint8))
                    nc.sync.dma_start(d_dbgH[:, NT * HPW8:2 * NT * HPW8],
                                      lo[:].bitcast(dt.uint8))

                # ---- PV: c-major ring; DoubleRow pairs, 3 passes ----
                with tc.tile_pool(name="pvps", bufs=4,
                                  space="PSUM") as psumv:
                    def emit_pv(g, c):
                        t0 = HT * g
                        IPf8 = IP[g][:].bitcast(dt.float8e4).rearrange(
                            "p (t i b) -> p t i b", t=NT, i=GW)
                        ps = psumv.tile([128, HPW8], dt.float32, tag="pvps",
                                        name=f"pv{g}_{c}")
                        for s in range(NT // 2):
                            lp = IPf8[:, 2 * s:2 * s + 2,
                                      128 * c:128 * (c + 1), 0]
                            ld = IPf8[:, 2 * s:2 * s + 2,
                                      128 * c:128 * (c + 1), 1]
                            rh = hiv[:, 2 * s:2 * s + 2, :]
                            rl = lov[:, 2 * s:2 * s + 2, :]
                            first = (s == 0)
                            last = (s == NT // 2 - 1)
                            for (lhsT, rhs, st, sp) in (
                                    (lp, rh, first, False),
                                    (lp, rl, False, False),
                                    (ld, rh, False, last)):
                                # start=True marks the whole 2KB psum bank
                                # pending-zero, so only the bank-leading
                                # chunk (f0 = 0 / 512) may set it; the
                                # companion chunk's first touch then WRITES
                                # (pending) rather than accumulating.
                                for f0, f1 in ((0, 256), (256, 512),
                                               (512, 768), (768, 769)):
                                    nc.tensor.matmul(
                                        ps[:, f0:f1], lhsT, rhs[:, :, f0:f1],
                                        start=st and f0 in (0, 512),
                                        stop=sp,
                                        perf_mode=PM.DoubleRow)
                        rz = p3.tile([128, 1], dt.float32, tag="rz",
                                     name=f"rz{g}_{c}")
                        nc.vector.reciprocal(rz[:], ps[:, F:F + 1])
                        if dbg and g == 0 and c == 0:
                            dps = opool.tile([128, HPW8], dt.float32,
                                             tag="dps")
                            nc.vector.tensor_copy(dps[:], ps[:])
                            nc.sync.dma_start(d_dbgPS.ap(), dps[:])
                            nc.sync.dma_start(d_dbgIA.ap(), IP[0][:])
                        ob = opool.tile([128, F], dt.float32, tag="ob")
                        nc.scalar.activation(ob[:], ps[:, 0:F], AF.Copy,
                                             scale=rz[:])
                        if has_bias:
                            nc.gpsimd.tensor_tensor(ob[:], ob[:], bias_bc[:],
                                                    op=OP.add)
                        vout = d_out.ap().rearrange(
                            "(t p) f -> t p f", t=NT, p=128)
                        dst = vout[t0:t0 + HT, 16 * c:16 * (c + 1),
                                   :].rearrange("t q f -> q t f")
                        nc.sync.dma_start(dst, ob[:])

                    for c in range(CHG):
                        build_slot(1, c)
                        emit_pv(0, c)
                    for k in range(HT, HT + 2):
                        build_slot(1, k)
                    for c in range(CHG):
                        emit_pv(1, c)

    nc.compile()
    return nc


def _get_program(has_bias: bool):
    import os
    key = ("prog", has_bias, os.environ.get("BMGA_DEBUG", "0"))
    if key not in _CACHE:
        _CACHE[key] = _build(has_bias)
    return _CACHE[key]


def kernel(h, adj, w, a_src, a_dst, bias):
    import ml_dtypes
    from concourse.bass_utils import run_bass_kernel_spmd

    BF = ml_dtypes.bfloat16
    h = np.asarray(h, dtype=np.float32)
    adj = np.asarray(adj).astype(bool)
    w = np.asarray(w, dtype=np.float32)
    a_src = np.asarray(a_src, dtype=np.float32).reshape(H, F)
    a_dst = np.asarray(a_dst, dtype=np.float32).reshape(H, F)
    bias = np.asarray(bias, dtype=np.float32).reshape(F)
    has_bias = bool(np.any(bias))

    nc = _get_program(has_bias)

    perm = _chip_perm()
    eye = np.eye(N, dtype=bool)
    wT8 = [np.ascontiguousarray(w[hd].astype(BF)) for hd in range(H)]
    in_maps = []
    for core in range(NCORES):
        b, hd = core // H, core % H
        keepT = (adj[b] | eye).T
        adjT = np.ascontiguousarray(
            keepT[:, perm]).astype(np.uint16) * np.uint16(0xFFFF)
        m = {
            "hT": np.ascontiguousarray(h[b].T.astype(BF)),
            "w": wT8[hd],
            "adjT": adjT,
            "a_src": a_src[hd],
            "a_dst": a_dst[hd],
        }
        if has_bias:
            m["bias"] = bias
        in_maps.append(m)

    res = run_bass_kernel_spmd(nc, in_maps, list(range(NCORES)))
    out = np.empty((BS, H, N, F), dtype=np.float32)
    for core in range(NCORES):
        b, hd = core // H, core % H
        out[b, hd] = res.results[core]["out"]
    return out


# revision 40
# speedup vs baseline: 1.0066x; 1.0066x over previous
"""BatchMultiHeadGraphAttention TRN2 kernel (fp8 DoubleRow PV).

Reference computation (per batch b, head h):
    h_prime = h[b] @ w[h]                          # [n, f]
    t = tanh(h_prime)
    src = t @ a_src[h];  dst = t @ a_dst[h]        # [n]
    s[i, j] = leaky_relu(src[i] + dst[j], 0.2)
    s = where(adj[b] | eye, s, -inf)
    attn = softmax(s, axis=-1)
    out[b, h] = attn @ h_prime + bias

Sharding: 8 cores, one (b, h) slab per core (bs=4 x H=2).

Algebra: exp(lrelu(s)) = v_j * max(u_i, p_i * r_j) with u=exp(src),
v=exp(dst), p=exp(0.2 src), r=exp(-0.8 dst).  The v_j factor is folded
into the value matrix (hpz = v_j * [h_prime | 1], the extra column
computing the softmax denominator Z), so softmax(s) @ h_prime =
(P @ hpz)[:, :F] / (P @ hpz)[:, F] with P[j, i] = mask * max(u_i, p_i r_j).

fp8 PV: P and hpz are held as e4m3 hi+residual pairs and the PV matmul
runs in MatmulPerfMode.DoubleRow (2 contraction tiles per instruction at
0.5 cycles/row -> 4x bf16 PE throughput).  Three DoubleRow passes,
P8@hi + P8@lo + dP8@hi, recover bf16-level accuracy (2.8e-3 max rel):
per-row scales c_i = 216/max(u_i, p_i*rmax_bound) (softmax-invariant)
put P in e4m3 range, a global g=4 scale (softmax-invariant, folded so
the hp scale is exactly exp(dst)) puts hpz in range.

Host-side prep does all layout work: h arrives pre-transposed in bf16
(hT), w in bf16, and the adjacency arrives transposed, diagonal-ORed,
column-permuted to the on-chip i order, and encoded as u16 0xFFFF/0 --
so masking P8/dP8 is a single 2x-mode bitwise-AND over a u16 plane that
holds both fp8 planes byte-interleaved.

On-chip i order within group g is (p, t)-major: chip i = 128*HT*g +
p*HT + t  <->  logical i = 128*(HT*g + t) + p.  This makes the
broadcast u/p rows plain contiguous DMAs; only the final output DMA
un-permutes (same descriptor count).
"""

import numpy as np

BS, N, H, F = 4, 2048, 2, 768
NCORES = 8
NT = N // 128          # 16 j tiles (tau)
KT = F // 128          # 6 k tiles
NG = 2                 # i groups
GW = N // NG           # 1024
HT = NT // NG          # 8 taus per group
CHG = GW // 128        # 8 i-chunks per group
HPW8 = 772             # fp8 value row stride (768 + Z col + pad)
LN216 = float(np.log(216.0))
LNRB = float(np.log(64.0))   # upper bound on ln(max_j r_j); true <= ln(18)

_CACHE = {}


def _chip_perm():
    """chip i -> logical i permutation (length N)."""
    perm = np.empty(N, np.int64)
    for g in range(NG):
        ci = np.arange(GW)
        perm[g * GW + ci] = 128 * (HT * g + ci % HT) + ci // HT
    return perm


def _build(has_bias: bool):
    import os
    import concourse.bass as bass
    import concourse.mybir as mybir
    import concourse.tile as tile
    from concourse import bacc

    dt = mybir.dt
    AF = mybir.ActivationFunctionType
    OP = mybir.AluOpType
    PM = mybir.MatmulPerfMode
    dbg = bool(int(os.environ.get("BMGA_DEBUG", "0")))

    nc = bacc.Bacc("TRN2", target_bir_lowering=False, debug=False,
                   num_devices=NCORES)

    d_hT = nc.dram_tensor("hT", [F, N], dt.bfloat16, kind="ExternalInput")
    d_w = nc.dram_tensor("w", [F, F], dt.bfloat16, kind="ExternalInput")
    d_adjT = nc.dram_tensor("adjT", [N, N], dt.uint16, kind="ExternalInput")
    d_asrc = nc.dram_tensor("a_src", [F], dt.float32, kind="ExternalInput")
    d_adst = nc.dram_tensor("a_dst", [F], dt.float32, kind="ExternalInput")
    if has_bias:
        d_bias = nc.dram_tensor("bias", [F], dt.float32, kind="ExternalInput")
    d_out = nc.dram_tensor("out", [N, F], dt.float32, kind="ExternalOutput")
    if dbg:
        d_dbgc = nc.dram_tensor("dbgc", [128, 4 * NT], dt.float32,
                                kind="ExternalOutput")
        d_dbgPS = nc.dram_tensor("dbgPS", [128, HPW8], dt.float32,
                                 kind="ExternalOutput")
        d_dbgH = nc.dram_tensor("dbgH", [128, 2 * NT * HPW8], dt.uint8,
                                kind="ExternalOutput")
        d_dbgIA = nc.dram_tensor("dbgIA", [128, NT * GW], dt.uint16,
                                 kind="ExternalOutput")

    with tile.TileContext(nc) as tc:
        with tc.tile_pool(name="const", bufs=1) as cpool, \
             tc.tile_pool(name="persist", bufs=1) as pp:
            if has_bias:
                bias_row = cpool.tile([1, F], dt.float32, tag="bias_row")
                nc.sync.dma_start(bias_row[:],
                                  d_bias.ap().rearrange("(o f) -> o f", o=1))
                bias_bc = cpool.tile([128, F], dt.float32, tag="bias_bc")
                nc.gpsimd.partition_broadcast(bias_bc[:], bias_row[:])

            # persistent: fp8 value planes, per-tau j columns, i broadcasts,
            # masked packed (P8, dP8) planes per group
            hi = pp.tile([128, NT * HPW8], dt.float8e4, tag="hi")
            lo = pp.tile([128, NT * HPW8], dt.float8e4, tag="lo")
            src_col = pp.tile([128, NT], dt.float32, tag="src_col")
            dst_col = pp.tile([128, NT], dt.float32, tag="dst_col")
            r_col = pp.tile([128, NT], dt.float32, tag="r_col")
            v_col = pp.tile([128, NT], dt.float32, tag="v_col")
            u_bc = pp.tile([128, N], dt.bfloat16, tag="u_bc")
            p_bc = pp.tile([128, N], dt.bfloat16, tag="p_bc")
            IP = [pp.tile([128, NT * GW], dt.uint16, tag=f"IP{g}",
                          name=f"IP{g}")
                  for g in range(NG)]

            hiv = hi[:].rearrange("p (t f) -> p t f", t=NT)
            lov = lo[:].rearrange("p (t f) -> p t f", t=NT)

            # ---- pools (whole-kernel scope except psum) ----
            with tc.tile_pool(name="ph1", bufs=1) as hpool, \
                 tc.tile_pool(name="ph2", bufs=3) as tpool, \
                 tc.tile_pool(name="scr2", bufs=2) as scrp, \
                 tc.tile_pool(name="hvp", bufs=2) as hvp, \
                 tc.tile_pool(name="p3", bufs=2) as p3, \
                 tc.tile_pool(name="pgm", bufs=6) as mpool, \
                 tc.tile_pool(name="pgsS", bufs=4) as spoolS, \
                 tc.tile_pool(name="pgsD", bufs=4) as spoolD, \
                 tc.tile_pool(name="pgo", bufs=2) as opool:
                # a_src/a_dst rows first: the dots need the broadcasts early
                asrc_row = hpool.tile([1, F], dt.float32, tag="asrc_row")
                nc.sync.dma_start(asrc_row[:],
                                  d_asrc.ap().rearrange("(o f) -> o f", o=1))
                adst_row = hpool.tile([1, F], dt.float32, tag="adst_row")
                nc.sync.dma_start(adst_row[:],
                                  d_adst.ap().rearrange("(o f) -> o f", o=1))
                asrc_bc = hpool.tile([128, F], dt.float32, tag="asrc_bc")
                nc.gpsimd.partition_broadcast(asrc_bc[:], asrc_row[:])
                adst_bc = hpool.tile([128, F], dt.float32, tag="adst_bc")
                nc.gpsimd.partition_broadcast(adst_bc[:], adst_row[:])
                wr = [hpool.tile([128, F], dt.bfloat16, tag=f"wr{k}",
                                 name=f"wr{k}") for k in range(KT)]
                hTt = [hpool.tile([128, N], dt.bfloat16, tag=f"hT{k}",
                                  name=f"hT{k}") for k in range(KT)]
                # k-interleaved w / hT-chunk loads: the k=0 matmul of tau 0
                # can start after the first (w, hT) pair lands
                for k in range(KT):
                    nc.sync.dma_start(wr[k][:], d_w[128 * k:128 * (k + 1), :])
                    nc.sync.dma_start(
                        hTt[k][:, 0:512],
                        d_hT[128 * k:128 * (k + 1), 0:512])
                for chnk in range(1, 4):
                    for k in range(KT):
                        nc.sync.dma_start(
                            hTt[k][:, 512 * chnk:512 * (chnk + 1)],
                            d_hT[128 * k:128 * (k + 1),
                                 512 * chnk:512 * (chnk + 1)])

                Smap, Sdmap, Mmap = {}, {}, {}

                def emit_phase2(tau, psum2):
                    ps = psum2.tile([128, F], dt.float32, tag="hpps")
                    for k in range(KT):
                        lhsT = hTt[k][:, 128 * tau:128 * (tau + 1)]
                        nc.tensor.matmul(ps[:, 0:512], lhsT, wr[k][:, 0:512],
                                         start=(k == 0), stop=(k == KT - 1))
                        nc.tensor.matmul(ps[:, 512:F], lhsT, wr[k][:, 512:F],
                                         start=(k == 0), stop=(k == KT - 1))
                    tnh = tpool.tile([128, F], dt.bfloat16, tag="tnh")
                    nc.scalar.activation(tnh[:], ps[:], AF.Tanh)
                    scrd = scrp.tile([128, F], dt.bfloat16, tag="scr")
                    nc.vector.scalar_tensor_tensor(
                        scrd[:], tnh[:], 1.0, adst_bc[:],
                        op0=OP.mult, op1=OP.mult,
                        accum_out=dst_col[:, tau:tau + 1])
                    nc.scalar.activation(r_col[:, tau:tau + 1],
                                         dst_col[:, tau:tau + 1],
                                         AF.Exp, scale=-0.8)
                    nc.scalar.activation(v_col[:, tau:tau + 1],
                                         dst_col[:, tau:tau + 1], AF.Exp)
                    # value planes: hi = fp8(v * h_prime), lo = residual
                    # (hpv staged in f32 so the lo subtract can run on Pool)
                    hslc = hi[:, HPW8 * tau:HPW8 * tau + F]
                    nc.scalar.activation(hslc, ps[:], AF.Copy,
                                         scale=v_col[:, tau:tau + 1])
                    hpv = hvp.tile([128, F], dt.float32, tag="hpv")
                    nc.scalar.activation(hpv[:], ps[:], AF.Copy,
                                         scale=v_col[:, tau:tau + 1])
                    nc.gpsimd.tensor_tensor(
                        lo[:, HPW8 * tau:HPW8 * tau + F],
                        hpv[:], hslc, op=OP.subtract)
                    scrs = scrp.tile([128, F], dt.bfloat16, tag="scr")
                    nc.vector.scalar_tensor_tensor(
                        scrs[:], tnh[:], 1.0, asrc_bc[:],
                        op0=OP.mult, op1=OP.mult,
                        accum_out=src_col[:, tau:tau + 1])
                    if tau % 4 == 3:
                        hz = hiv[:, tau - 3:tau + 1, F]
                        nc.scalar.activation(hz, v_col[:, tau - 3:tau + 1],
                                             AF.Copy)
                        nc.vector.tensor_tensor(
                            lov[:, tau - 3:tau + 1, F],
                            v_col[:, tau - 3:tau + 1], hz, op=OP.subtract)

                def emit_iside(g):
                    # c_i = 216/max(u_i, p_i*rmax_bound) in log domain
                    t0 = HT * g
                    sg = src_col[:, t0:t0 + HT]
                    s3 = p3.tile([128, HT], dt.float32, tag="s3")
                    nc.vector.tensor_scalar(s3[:], sg, 0.2, None, op0=OP.mult)
                    s2 = p3.tile([128, HT], dt.float32, tag="s2")
                    nc.vector.tensor_scalar(s2[:], s3[:], LNRB, None,
                                            op0=OP.add)
                    mxs = p3.tile([128, HT], dt.float32, tag="mxs")
                    nc.vector.tensor_tensor(mxs[:], sg, s2[:], op=OP.max)
                    nc.vector.tensor_scalar(mxs[:], mxs[:], LN216, None,
                                            op0=OP.subtract)
                    eu = p3.tile([128, HT], dt.float32, tag="eu")
                    nc.vector.tensor_tensor(eu[:], sg, mxs[:],
                                            op=OP.subtract)
                    ep = p3.tile([128, HT], dt.float32, tag="ep")
                    nc.vector.tensor_tensor(ep[:], s3[:], mxs[:],
                                            op=OP.subtract)
                    ux = p3.tile([128, HT], dt.bfloat16, tag="ux")
                    nc.scalar.activation(ux[:], eu[:], AF.Exp)
                    px = p3.tile([128, HT], dt.bfloat16, tag="px")
                    nc.scalar.activation(px[:], ep[:], AF.Exp)
                    u_row = p3.tile([1, GW], dt.bfloat16, tag="u_row")
                    nc.sync.dma_start(
                        u_row[:].rearrange("o (p t) -> o p t", p=128), ux[:])
                    p_row = p3.tile([1, GW], dt.bfloat16, tag="p_row")
                    nc.sync.dma_start(
                        p_row[:].rearrange("o (p t) -> o p t", p=128), px[:])
                    nc.gpsimd.partition_broadcast(
                        u_bc[:, GW * g:GW * (g + 1)], u_row[:])
                    nc.gpsimd.partition_broadcast(
                        p_bc[:, GW * g:GW * (g + 1)], p_row[:])

                def emit_smax(g, tau):
                    mtile = mpool.tile([128, GW], dt.uint16, tag="mask")
                    nc.sync.dma_start(
                        mtile[:],
                        d_adjT[128 * tau:128 * (tau + 1),
                               GW * g:GW * (g + 1)])
                    Mmap[(g, tau)] = mtile
                    S = spoolS.tile([128, GW], dt.float32, tag="S")
                    nc.vector.scalar_tensor_tensor(
                        S[:], p_bc[:, GW * g:GW * (g + 1)],
                        r_col[:, tau:tau + 1],
                        u_bc[:, GW * g:GW * (g + 1)],
                        op0=OP.mult, op1=OP.max)
                    Smap[(g, tau)] = S

                def emit_s8(g, tau):
                    Sd = spoolD.tile([128, GW], dt.uint16, tag="Sd")
                    Sdmap[(g, tau)] = Sd
                    s8v = Sd[:].bitcast(dt.float8e4)[:, 0:2 * GW:2]
                    nc.scalar.activation(s8v, Smap[(g, tau)][:], AF.Copy)

                def emit_ds8(g, tau):
                    Sd = Sdmap[(g, tau)]
                    s8v = Sd[:].bitcast(dt.float8e4)[:, 0:2 * GW:2]
                    ds8v = Sd[:].bitcast(dt.float8e4)[:, 1:2 * GW:2]
                    ti = g * NT + tau
                    ds8eng = nc.vector if ti % 4 == 3 else nc.gpsimd
                    ds8eng.tensor_tensor(ds8v, Smap[(g, tau)][:], s8v,
                                         op=OP.subtract)

                def emit_and(g, tau):
                    nc.vector.tensor_tensor(
                        IP[g][:].bitcast(dt.uint32)[
                            :, GW // 2 * tau:GW // 2 * (tau + 1)],
                        Mmap[(g, tau)][:].bitcast(dt.uint32),
                        Sdmap[(g, tau)][:].bitcast(dt.uint32),
                        op=OP.bitwise_and)
                    del Smap[(g, tau)], Sdmap[(g, tau)], Mmap[(g, tau)]

                # per-slot build tau schedules: g0 trails its last 4 taus
                # into the PV(g0) window so the phase-2 tail window sheds
                # DVE/Pool/ACT work
                SCHED = {0: [(0, 1), (2, 3), (4, 5), (6, 7), (8, 9),
                             (10, 11), (12,), (13,), (14,), (15,)],
                         1: [(2 * k, 2 * k + 1) for k in range(HT)]}

                def build_slot(g, k):
                    # software-pipelined build: smax@k, S8+dS8@k+1,
                    # AND@k+2
                    if k == 0:
                        emit_iside(g)
                    sched = SCHED[g]
                    for stage, d in ((emit_smax, 0), (emit_s8, 1),
                                     (emit_ds8, 1), (emit_and, 2)):
                        kk = k - d
                        if 0 <= kk < len(sched):
                            for tau in sched[kk]:
                                stage(g, tau)

                with tc.tile_pool(name="ph2ps", bufs=3,
                                  space="PSUM") as psum2:
                    for tau in range(NT):
                        emit_phase2(tau, psum2)
                        if tau >= NT - HT:
                            build_slot(0, tau - (NT - HT))
                    for k in range(HT, HT + 2):
                        build_slot(0, k)
                # remaining g0 build slots trail into the PV block

                if dbg:
                    nc.sync.dma_start(d_dbgc[:, 0:NT], src_col[:])
                    nc.sync.dma_start(d_dbgc[:, NT:2 * NT], dst_col[:])
                    nc.sync.dma_start(d_dbgc[:, 2 * NT:3 * NT], r_col[:])
                    nc.sync.dma_start(d_dbgc[:, 3 * NT:4 * NT], v_col[:])
                    nc.sync.dma_start(d_dbgH[:, 0:NT * HPW8],
                                      hi[:].bitcast(dt.uint8))
                    nc.sync.dma_start(d_dbgH[:, NT * HPW8:2 * NT * HPW8],
                                      lo[:].bitcast(dt.uint8))

                # ---- PV: c-major ring; DoubleRow pairs, 3 passes ----
                with tc.tile_pool(name="pvps", bufs=4,
                                  space="PSUM") as psumv:
                    def emit_pv(g, c):
                        t0 = HT * g
                        IPf8 = IP[g][:].bitcast(dt.float8e4).rearrange(
                            "p (t i b) -> p t i b", t=NT, i=GW)
                        ps = psumv.tile([128, HPW8], dt.float32, tag="pvps",
                                        name=f"pv{g}_{c}")
                        for s in range(NT // 2):
                            lp = IPf8[:, 2 * s:2 * s + 2,
                                      128 * c:128 * (c + 1), 0]
                            ld = IPf8[:, 2 * s:2 * s + 2,
                                      128 * c:128 * (c + 1), 1]
                            rh = hiv[:, 2 * s:2 * s + 2, :]
                            rl = lov[:, 2 * s:2 * s + 2, :]
                            first = (s == 0)
                            last = (s == NT // 2 - 1)
                            for (lhsT, rhs, st, sp) in (
                                    (lp, rh, first, False),
                                    (lp, rl, False, False),
                                    (ld, rh, False, last)):
                                # start=True marks the whole 2KB psum bank
                                # pending-zero, so only the bank-leading
                                # chunk (f0 = 0 / 512) may set it; the
                                # companion chunk's first touch then WRITES
                                # (pending) rather than accumulating.
                                for f0, f1 in ((0, 256), (256, 512),
                                               (512, 768), (768, 769)):
                                    nc.tensor.matmul(
                                        ps[:, f0:f1], lhsT, rhs[:, :, f0:f1],
                                        start=st and f0 in (0, 512),
                                        stop=sp,
                                        perf_mode=PM.DoubleRow)
                        rz = p3.tile([128, 1], dt.float32, tag="rz",
                                     name=f"rz{g}_{c}")
                        nc.vector.reciprocal(rz[:], ps[:, F:F + 1])
                        if dbg and g == 0 and c == 0:
                            dps = opool.tile([128, HPW8], dt.float32,
                                             tag="dps")
                            nc.vector.tensor_copy(dps[:], ps[:])
                            nc.sync.dma_start(d_dbgPS.ap(), dps[:])
                            nc.sync.dma_start(d_dbgIA.ap(), IP[0][:])
                        ob = opool.tile([128, F], dt.float32, tag="ob")
                        nc.scalar.activation(ob[:], ps[:, 0:F], AF.Copy,
                                             scale=rz[:])
                        if has_bias:
                            nc.gpsimd.tensor_tensor(ob[:], ob[:], bias_bc[:],
                                                    op=OP.add)
                        vout = d_out.ap().rearrange(
                            "(t p) f -> t p f", t=NT, p=128)
                        dst = vout[t0:t0 + HT, 16 * c:16 * (c + 1),
                                   :].rearrange("t q f -> q t f")
                        nc.sync.dma_start(dst, ob[:])

                    for c in range(CHG):
                        if c < 4:
                            build_slot(0, HT + 2 + c)
                        build_slot(1, c)
                        emit_pv(0, c)
                    for k in range(HT, HT + 2):
                        build_slot(1, k)
                    for c in range(CHG):
                        emit_pv(1, c)

    nc.compile()
    return nc


def _get_program(has_bias: bool):
    import os
    key = ("prog", has_bias, os.environ.get("BMGA_DEBUG", "0"))
    if key not in _CACHE:
        _CACHE[key] = _build(has_bias)
    return _CACHE[key]


def kernel(h, adj, w, a_src, a_dst, bias):
    import ml_dtypes
    from concourse.bass_utils import run_bass_kernel_spmd

    BF = ml_dtypes.bfloat16
    h = np.asarray(h, dtype=np.float32)
    adj = np.asarray(adj).astype(bool)
    w = np.asarray(w, dtype=np.float32)
    a_src = np.asarray(a_src, dtype=np.float32).reshape(H, F)
    a_dst = np.asarray(a_dst, dtype=np.float32).reshape(H, F)
    bias = np.asarray(bias, dtype=np.float32).reshape(F)
    has_bias = bool(np.any(bias))

    nc = _get_program(has_bias)

    perm = _chip_perm()
    eye = np.eye(N, dtype=bool)
    wT8 = [np.ascontiguousarray(w[hd].astype(BF)) for hd in range(H)]
    in_maps = []
    for core in range(NCORES):
        b, hd = core // H, core % H
        keepT = (adj[b] | eye).T
        adjT = np.ascontiguousarray(
            keepT[:, perm]).astype(np.uint16) * np.uint16(0xFFFF)
        m = {
            "hT": np.ascontiguousarray(h[b].T.astype(BF)),
            "w": wT8[hd],
            "adjT": adjT,
            "a_src": a_src[hd],
            "a_dst": a_dst[hd],
        }
        if has_bias:
            m["bias"] = bias
        in_maps.append(m)

    res = run_bass_kernel_spmd(nc, in_maps, list(range(NCORES)))
    out = np.empty((BS, H, N, F), dtype=np.float32)
    for core in range(NCORES):
        b, hd = core // H, core % H
        out[b, hd] = res.results[core]["out"]
    return out
